# revision 15
# baseline (speedup 1.0000x reference)
"""Trainium2 Bass kernel for GNN message-passing attention (B=64, N=2048, D=128).

Math (per reference):
    q     = e_i @ W_q.T                                  [B, D]
    k     = e_j @ W_k.T                                  [B, N, D]
    sigma = tanh(q[:, None, :] + k) @ omega              [B, N]
    es    = exp(sigma); den = sum_n es + 1e-9
    a     = imp_j * es / den
    A_l   = einsum('bn,bnd->bd', a, e_j @ W_v.T)
          = (einsum('bn,bnd->bd', imp*es, e_j) @ W_v.T) / den      (v never materialized)
    A_lk  = R_lk[None] * A_l[:, None, :]
    returns (A_l, A_lk, A_l)

Distribution: pure data-parallel over batch B across 8 NeuronCores (8 batches
per core), small weights replicated, no collectives.

Per-core dataflow (e-major):
  - DMA e_j[b] natural [n,d] tiles, f32->bf16 cast in SWDGE
  - PE transpose-mode -> e_jT [d,n] (bf16)
  - kT = W_kT.T @ e_jT   (bf16 matmul, W_kT stationary, 512-wide chunks)
  - ACT tanh(kT + q_b) with per-partition bias  -> TANH (f32)
  - sigma columns: matmul(lhsT=TANH-tile, rhs=omega) -> [n,1] per tile
  - ACT exp with accum_out -> es columns + per-partition partial denominator
  - w = es * impT (DVE) -> bf16
  - u[:,b] += EJ-tile.T @ w-col (PSUM accumulation over 16 tiles)
  Epilogue: denom via ones-matmul, reciprocal, scale u, A_lT = W_vT.T @ u,
  A_l via PE transpose, A_lk[b] = transpose(R_T * A_l[b]-col).
"""

import os
import sys

import numpy as np

sys.path.insert(0, "/opt/trn_rl_repo")

import ml_dtypes  # noqa: E402

BF16 = ml_dtypes.bfloat16

B, N, D = 64, 2048, 128
NCORES = 8
BL = B // NCORES  # 8 batches per core
T = N // 128  # 16 n-tiles per batch
CH = 4  # kT chunks of 512
CHW = N // CH  # 512

_CACHE = {}


def _build_nc():
    import concourse.bass as bass
    import concourse.tile as tile
    from concourse import mybir

    f32 = mybir.dt.float32
    bf16 = mybir.dt.bfloat16
    AF = mybir.ActivationFunctionType

    nc = bass.Bass("TRN2", target_bir_lowering=False, debug=False, num_devices=NCORES)

    # ---- I/O ----
    # e_j passed pre-cast to bf16 host-side: halves HBM traffic and enables
    # the xbar DMA-transpose path (2-byte dtypes only).
    ej_d = nc.dram_tensor("e_j", [BL, N, D], bf16, kind="ExternalInput")
    eiT_d = nc.dram_tensor("e_iT", [D, BL], f32, kind="ExternalInput")
    impT_d = nc.dram_tensor("impT", [128, T * BL], f32, kind="ExternalInput")
    wqT_d = nc.dram_tensor("W_qT", [D, D], f32, kind="ExternalInput")
    wkT_d = nc.dram_tensor("W_kT", [D, D], bf16, kind="ExternalInput")
    wvT_d = nc.dram_tensor("W_vT", [D, D], f32, kind="ExternalInput")
    rT_d = nc.dram_tensor("R_T", [D, D], f32, kind="ExternalInput")
    om_d = nc.dram_tensor("omega", [D, 1], bf16, kind="ExternalInput")
    onc_d = nc.dram_tensor("ones_col", [D, 1], f32, kind="ExternalInput")
    onr_d = nc.dram_tensor("ones_row", [1, D], f32, kind="ExternalInput")
    if32_d = nc.dram_tensor("I_f32", [D, D], f32, kind="ExternalInput")

    al_d = nc.dram_tensor("A_l", [BL, D], f32, kind="ExternalOutput")
    alk_d = nc.dram_tensor("A_lk", [BL, D, D], f32, kind="ExternalOutput")

    with tile.TileContext(nc) as tc:
        with (
            tc.tile_pool(name="const", bufs=1) as const,
            tc.tile_pool(name="ej", bufs=8) as ejp,
            tc.tile_pool(name="ejt", bufs=8) as ejtp,
            tc.tile_pool(name="tanh", bufs=4) as tanhp,
            tc.tile_pool(name="small", bufs=3) as smallp,
            tc.tile_pool(name="persist", bufs=1) as persist,
            tc.tile_pool(name="out", bufs=2) as outp,
            tc.tile_pool(name="pT", bufs=1, space="PSUM") as pTp,
            tc.tile_pool(name="pk", bufs=4, space="PSUM") as pkp,
            tc.tile_pool(name="ps", bufs=2, space="PSUM") as psp,
            tc.tile_pool(name="pu", bufs=1, space="PSUM") as pup,
        ):
            # ---- constants into SBUF ----
            def cload(dram, shape, dt, tag):
                t = const.tile(shape, dt, tag=tag)
                nc.sync.dma_start(t[:], dram[:])
                return t

            eiT = cload(eiT_d, [D, BL], f32, "eiT")
            wqT = cload(wqT_d, [D, D], f32, "wqT")
            wkT = cload(wkT_d, [D, D], bf16, "wkT")
            wvT = cload(wvT_d, [D, D], f32, "wvT")
            rT = cload(rT_d, [D, D], f32, "rT")
            om = cload(om_d, [D, 1], bf16, "om")
            onc = cload(onc_d, [D, 1], f32, "onc")
            onr = cload(onr_d, [1, D], f32, "onr")
            if32 = cload(if32_d, [D, D], f32, "if32")

            # impT pre-arranged host-side: impT[p, g, b] = imp[b, 16p+g]
            impT_sb = const.tile([128, T * BL], f32)
            nc.sync.dma_start(impT_sb[:], impT_d[:, :])
            impT3 = impT_sb[:].rearrange("p (t b) -> p t b", b=BL)

            # ---- q = e_i @ W_q.T, as columns qT [e, b] ----
            pq = psp.tile([D, BL], f32, tag="ps")
            nc.tensor.matmul(pq[:], wqT[:], eiT[:], start=True, stop=True)
            qT = persist.tile([D, BL], f32)
            nc.vector.tensor_copy(qT[:], pq[:])

            # expsum[:, b] = per-partition partial sum of exp(sigma) for batch b
            expsum = persist.tile([128, BL], f32)
            # u accumulator [d, b]
            pu = pup.tile([D, BL], f32)

            # Prologue: issue all input DMAs up front; everything stays
            # resident so the 8 per-batch compute chains pipeline freely.
            EJs, EJTs = [], []
            for b in range(BL):
                # EJ natural with n = 16*p + g mapping: each partition reads a
                # contiguous 4KB run from DRAM (128 descriptors vs 2048).
                # The u-reduction over n is permutation-invariant; sigma tiles
                # and imp use the same mapping.
                EJ = ejp.tile([128, T * D], bf16)
                nc.scalar.dma_start(
                    EJ[:].rearrange("p (g x) -> p g x", x=D),
                    ej_d.ap()[b].rearrange("(p g) x -> p g x", g=T),
                )
                EJs.append(EJ)
                # EJT via hardware xbar DMA-transpose: [d, n]
                EJT = ejtp.tile([128, N], bf16)
                nc.sync.dma_start_transpose(EJT[:], ej_d.ap()[b])
                EJTs.append(EJT)

            for b in range(BL):
                EJ, EJT = EJs[b], EJTs[b]

                # kT chunks + tanh(. + q_b); tanh stored bf16
                TANH = tanhp.tile([128, N], bf16)
                for c in range(CH):
                    pk = pkp.tile([128, CHW], f32, tag="pk")
                    nc.tensor.matmul(
                        pk[:],
                        wkT[:],
                        EJT[:, c * CHW : (c + 1) * CHW],
                        start=True,
                        stop=True,
                    )
                    nc.scalar.activation(
                        TANH[:, c * CHW : (c + 1) * CHW],
                        pk[:],
                        AF.Tanh,
                        bias=qT[:, b : b + 1],
                    )

                # sigma columns [n, 1] per tile -> ps [128, T].
                # Tile g gathers n = 16*p + g (strided lhsT) to match EJ.
                TANH3 = TANH[:].rearrange("e (p g) -> e g p", g=T)
                ps = psp.tile([128, T], f32, tag="ps")
                for t in range(T):
                    nc.tensor.matmul(
                        ps[:, t : t + 1],
                        TANH3[:, t, :],
                        om[:],
                        start=(t == 0),
                        stop=(t == T - 1),
                        skip_group_check=True,
                    )

                # exp + per-partition denominator partial
                es = smallp.tile([128, T], f32, tag="es")
                nc.scalar.activation(
                    es[:], ps[:], AF.Exp, accum_out=expsum[:, b : b + 1]
                )

                # w = es * imp (unnormalized attention weight), bf16 for u-matmul
                w = smallp.tile([128, T], bf16, tag="w")
                nc.vector.tensor_mul(w[:], es[:], impT3[:, :, b])

                # u[:, b] += EJ_tile.T @ w_col
                for t in range(T):
                    nc.tensor.matmul(
                        pu[:, b : b + 1],
                        EJ[:, t * 128 : (t + 1) * 128],
                        w[:, t : t + 1],
                        start=(b == 0 and t == 0),
                        stop=(b == BL - 1 and t == T - 1),
                        skip_group_check=True,
                    )

            # ---- epilogue ----
            # denom row [1, b] = sum_p expsum
            pden = pTp.tile([1, BL], f32, tag="pT")
            nc.tensor.matmul(pden[:], onc[:], expsum[:], start=True, stop=True)
            den = smallp.tile([1, BL], f32, tag="den")
            nc.vector.tensor_scalar_add(den[:], pden[:], 1e-9)
            recip = smallp.tile([1, BL], f32, tag="recip")
            nc.vector.reciprocal(recip[:], den[:])

            # broadcast recip down 128 partitions
            prb = pTp.tile([128, BL], f32, tag="pT")
            nc.tensor.matmul(prb[:], onr[:], recip[:], start=True, stop=True)
            rb = smallp.tile([128, BL], f32, tag="rb")
            nc.vector.tensor_copy(rb[:], prb[:])

            # scale u by 1/den while copying out of PSUM
            u = smallp.tile([D, BL], f32, tag="u")
            nc.vector.tensor_mul(u[:], pu[:], rb[:])

            # A_lT [l, b] = W_vT.T @ u
            pal = pkp.tile([D, BL], f32, tag="pk")
            nc.tensor.matmul(pal[:], wvT[:], u[:], start=True, stop=True)
            alT = persist.tile([D, BL], f32)
            nc.vector.tensor_copy(alT[:], pal[:])

            # A_l rows: transpose alT -> [b, l]
            palr = pTp.tile([BL, D], f32, tag="pT")
            nc.tensor.matmul(palr[:], alT[:], if32[:], is_transpose=True)
            alr = smallp.tile([BL, D], f32, tag="alr")
            nc.vector.tensor_copy(alr[:], palr[:])
            nc.sync.dma_start(al_d[:, :], alr[:])

            # A_lk[b] = transpose(R_T * A_l[b]-col)
            for b in range(BL):
                alkT = outp.tile([D, D], f32, tag="alkT")
                nc.vector.tensor_scalar_mul(alkT[:], rT[:], alT[:, b : b + 1])
                palk = pkp.tile([D, D], f32, tag="pk")
                nc.tensor.matmul(palk[:], alkT[:], if32[:], is_transpose=True)
                alk = outp.tile([D, D], f32, tag="alk")
                nc.vector.tensor_copy(alk[:], palk[:])
                nc.gpsimd.dma_start(alk_d.ap()[b], alk[:])

    _split_multi_waits(nc)
    return nc


def _split_multi_waits(nc):
    """This walrus build allows at most ONE sync-wait per instruction
    (setupSyncWait: 'Too many sync wait commands'). Tile attaches one wait
    per dependency semaphore. Split the extras into standalone
    EventSemaphore instructions on the same engine queue, inserted just
    before the instruction — same mechanism Tile's own barriers use."""
    from concourse import mybir

    for fn in nc.m.functions:
        for blk in fn.blocks:
            out = []
            for i in blk.instructions:
                si = getattr(i, "sync_info", None)
                if si is not None and len(si.on_wait) > 1:
                    for j, w in enumerate(si.on_wait[:-1]):
                        out.append(
                            mybir.InstEventSemaphore(
                                name=f"{i.name}_prewait{j}",
                                engine=i.engine,
                                ins=[],
                                outs=[],
                                sync_info=mybir.SyncInfo(on_wait=[w], on_update=[]),
                            )
                        )
                    i.sync_info = mybir.SyncInfo(
                        on_wait=[si.on_wait[-1]], on_update=si.on_update
                    )
                out.append(i)
            try:
                blk.instructions = out
            except Exception:
                blk.instructions.clear()
                blk.instructions.extend(out)


def _get_nc():
    if "nc" not in _CACHE:
        _CACHE["nc"] = _build_nc()
    return _CACHE["nc"]


def kernel(e_i, e_j, imp_j, R_lk, W_q, W_k, W_v, omega):
    from concourse.bass_utils import run_bass_kernel_spmd

    e_i = np.asarray(e_i, np.float32)
    e_j = np.asarray(e_j, np.float32)
    imp_j = np.asarray(imp_j, np.float32)

    wqT = np.ascontiguousarray(np.asarray(W_q, np.float32).T)
    wkT = np.ascontiguousarray(np.asarray(W_k, np.float32).T).astype(BF16)
    wvT = np.ascontiguousarray(np.asarray(W_v, np.float32).T)
    rT = np.ascontiguousarray(np.asarray(R_lk, np.float32).T)
    om = np.asarray(omega, np.float32).reshape(D, 1).astype(BF16)
    onc = np.ones((D, 1), np.float32)
    onr = np.ones((1, D), np.float32)
    if32 = np.eye(D, dtype=np.float32)

    in_maps = []
    for c in range(NCORES):
        sl = slice(c * BL, (c + 1) * BL)
        in_maps.append(
            {
                "e_j": np.ascontiguousarray(e_j[sl]).astype(BF16),
                "e_iT": np.ascontiguousarray(e_i[sl].T),
                "impT": np.ascontiguousarray(
                    imp_j[sl].reshape(BL, 128, T).transpose(1, 2, 0).reshape(128, T * BL)
                ),
                "W_qT": wqT,
                "W_kT": wkT,
                "W_vT": wvT,
                "R_T": rT,
                "omega": om,
                "ones_col": onc,
                "ones_row": onr,
                "I_f32": if32,
            }
        )

    nc = _get_nc()
    trace = bool(int(os.environ.get("ATHENA_TRACE", "0")))
    res = run_bass_kernel_spmd(
        nc, in_maps, core_ids=list(range(NCORES)), trace=trace
    )
    _CACHE["last_results"] = res

    A_l = np.concatenate([res.results[c]["A_l"] for c in range(NCORES)], axis=0)
    A_lk = np.concatenate([res.results[c]["A_lk"] for c in range(NCORES)], axis=0)
    return A_l, A_lk, A_l


# revision 19
# speedup vs baseline: 1.3016x; 1.3016x over previous
"""Trainium2 Bass kernel for GNN message-passing attention (B=64, N=2048, D=128).

Math (per reference):
    q     = e_i @ W_q.T                                  [B, D]
    k     = e_j @ W_k.T                                  [B, N, D]
    sigma = tanh(q[:, None, :] + k) @ omega              [B, N]
    es    = exp(sigma); den = sum_n es + 1e-9
    a     = imp_j * es / den
    A_l   = einsum('bn,bnd->bd', a, e_j @ W_v.T)
          = (einsum('bn,bnd->bd', imp*es, e_j) @ W_v.T) / den      (v never materialized)
    A_lk  = R_lk[None] * A_l[:, None, :]
    returns (A_l, A_lk, A_l)

Distribution: pure data-parallel over batch B across 8 NeuronCores (8 batches
per core), small weights replicated, no collectives.

Per-core dataflow (e-major):
  - DMA e_j[b] natural [n,d] tiles, f32->bf16 cast in SWDGE
  - PE transpose-mode -> e_jT [d,n] (bf16)
  - kT = W_kT.T @ e_jT   (bf16 matmul, W_kT stationary, 512-wide chunks)
  - ACT tanh(kT + q_b) with per-partition bias  -> TANH (f32)
  - sigma columns: matmul(lhsT=TANH-tile, rhs=omega) -> [n,1] per tile
  - ACT exp with accum_out -> es columns + per-partition partial denominator
  - w = es * impT (DVE) -> bf16
  - u[:,b] += EJ-tile.T @ w-col (PSUM accumulation over 16 tiles)
  Epilogue: denom via ones-matmul, reciprocal, scale u, A_lT = W_vT.T @ u,
  A_l via PE transpose, A_lk[b] = transpose(R_T * A_l[b]-col).
"""

import os
import sys

import numpy as np

sys.path.insert(0, "/opt/trn_rl_repo")

import ml_dtypes  # noqa: E402

BF16 = ml_dtypes.bfloat16

B, N, D = 64, 2048, 128
NCORES = 8
BL = B // NCORES  # 8 batches per core
T = N // 128  # 16 n-tiles per batch
CH = 4  # kT chunks of 512
CHW = N // CH  # 512

_CACHE = {}


def _build_nc():
    import concourse.bass as bass
    import concourse.tile as tile
    from concourse import mybir

    f32 = mybir.dt.float32
    bf16 = mybir.dt.bfloat16
    AF = mybir.ActivationFunctionType

    nc = bass.Bass("TRN2", target_bir_lowering=False, debug=False, num_devices=NCORES)

    # ---- I/O ----
    # e_j passed pre-cast to bf16 host-side: halves HBM traffic and enables
    # the xbar DMA-transpose path (2-byte dtypes only).
    ej_d = nc.dram_tensor("e_j", [BL, N, D], bf16, kind="ExternalInput")
    # all small f32 constants packed into one tensor, one DMA:
    # cols: eiT[0:8] wqT[8:136] wvT[136:264] rT[264:392] if32[392:520]
    #       onc[520:521] impT[521:649]
    cf_d = nc.dram_tensor("constf", [128, 649], f32, kind="ExternalInput")
    cb_d = nc.dram_tensor("constb", [128, 129], bf16, kind="ExternalInput")
    onr_d = nc.dram_tensor("ones_row", [1, D], f32, kind="ExternalInput")

    al_d = nc.dram_tensor("A_l", [BL, D], f32, kind="ExternalOutput")
    alk_d = nc.dram_tensor("A_lk", [BL, D, D], f32, kind="ExternalOutput")

    with tile.TileContext(nc) as tc:
        with (
            tc.tile_pool(name="const", bufs=1) as const,
            tc.tile_pool(name="ej", bufs=8) as ejp,
            tc.tile_pool(name="ejt", bufs=8) as ejtp,
            tc.tile_pool(name="tanh", bufs=4) as tanhp,
            tc.tile_pool(name="small", bufs=3) as smallp,
            tc.tile_pool(name="persist", bufs=1) as persist,
            tc.tile_pool(name="out", bufs=2) as outp,
            tc.tile_pool(name="pT", bufs=1, space="PSUM") as pTp,
            tc.tile_pool(name="pk", bufs=3, space="PSUM") as pkp,
            tc.tile_pool(name="ps", bufs=3, space="PSUM") as psp,
            tc.tile_pool(name="pu", bufs=1, space="PSUM") as pup,
        ):
            # ---- constants into SBUF: 3 DMAs total ----
            cf = const.tile([128, 649], f32, tag="cf")
            nc.sync.dma_start(cf[:], cf_d[:, :])
            cb = const.tile([128, 129], bf16, tag="cb")
            nc.sync.dma_start(cb[:], cb_d[:, :])
            onr = const.tile([1, D], f32, tag="onr")
            nc.sync.dma_start(onr[:], onr_d[:, :])

            eiT = cf[:, 0:8]
            wqT = cf[:, 8:136]
            wvT = cf[:, 136:264]
            rT = cf[:, 264:392]
            if32 = cf[:, 392:520]
            onc = cf[:, 520:521]
            impT3 = cf[:, 521:649].rearrange("p (t b) -> p t b", b=BL)
            wkT = cb[:, 0:128]
            om = cb[:, 128:129]

            # ---- q = e_i @ W_q.T, as columns qT [e, b] ----
            pq = psp.tile([D, BL], f32, tag="ps")
            nc.tensor.matmul(pq[:], wqT, eiT, start=True, stop=True)
            qT = persist.tile([D, BL], f32)
            nc.vector.tensor_copy(qT[:], pq[:])

            # expsum[:, b] = per-partition partial sum of exp(sigma) for batch b
            expsum = persist.tile([128, BL], f32)
            # u accumulator [d, b]
            pu = pup.tile([D, BL], f32)

            for b in range(BL):
                # EJ natural with n = 16*p + g mapping: each partition reads a
                # contiguous 4KB run from DRAM (128 descriptors vs 2048).
                # The u-reduction over n is permutation-invariant; sigma tiles
                # and imp use the same mapping.
                EJ = ejp.tile([128, T * D], bf16)
                nc.scalar.dma_start(
                    EJ[:].rearrange("p (g x) -> p g x", x=D),
                    ej_d.ap()[b].rearrange("(p g) x -> p g x", g=T),
                )
                # EJT via hardware xbar DMA-transpose: [d, n]
                EJT = ejtp.tile([128, N], bf16)
                nc.sync.dma_start_transpose(EJT[:], ej_d.ap()[b])

                # kT chunks + tanh(. + q_b); tanh stored bf16
                TANH = tanhp.tile([128, N], bf16)
                for c in range(CH):
                    pk = pkp.tile([128, CHW], f32, tag="pk")
                    nc.tensor.matmul(
                        pk[:],
                        wkT,
                        EJT[:, c * CHW : (c + 1) * CHW],
                        start=True,
                        stop=True,
                    )
                    nc.scalar.activation(
                        TANH[:, c * CHW : (c + 1) * CHW],
                        pk[:],
                        AF.Tanh,
                        bias=qT[:, b : b + 1],
                    )

                # sigma columns [n, 1] per tile -> ps [128, T].
                # Tile g gathers n = 16*p + g (strided lhsT) to match EJ.
                TANH3 = TANH[:].rearrange("e (p g) -> e g p", g=T)
                ps = psp.tile([128, T], f32, tag="ps")
                for t in range(T):
                    nc.tensor.matmul(
                        ps[:, t : t + 1],
                        TANH3[:, t, :],
                        om,
                        start=(t == 0),
                        stop=(t == T - 1),
                        skip_group_check=True,
                    )

                # exp + per-partition denominator partial
                es = smallp.tile([128, T], f32, tag="es")
                nc.scalar.activation(
                    es[:], ps[:], AF.Exp, accum_out=expsum[:, b : b + 1]
                )

                # w = es * imp (unnormalized attention weight), bf16 for u-matmul
                w = smallp.tile([128, T], bf16, tag="w")
                nc.vector.tensor_mul(w[:], es[:], impT3[:, :, b])

                # u[:, b] += EJ_tile.T @ w_col
                for t in range(T):
                    nc.tensor.matmul(
                        pu[:, b : b + 1],
                        EJ[:, t * 128 : (t + 1) * 128],
                        w[:, t : t + 1],
                        start=(b == 0 and t == 0),
                        stop=(b == BL - 1 and t == T - 1),
                        skip_group_check=True,
                    )

            # ---- epilogue ----
            # denom row [1, b] = sum_p expsum
            pden = pTp.tile([1, BL], f32, tag="pT")
            nc.tensor.matmul(pden[:], onc, expsum[:], start=True, stop=True)
            den = smallp.tile([1, BL], f32, tag="den")
            nc.vector.tensor_scalar_add(den[:], pden[:], 1e-9)
            recip = smallp.tile([1, BL], f32, tag="recip")
            nc.vector.reciprocal(recip[:], den[:])

            # broadcast recip down 128 partitions
            prb = pTp.tile([128, BL], f32, tag="pT")
            nc.tensor.matmul(prb[:], onr[:], recip[:], start=True, stop=True)
            rb = smallp.tile([128, BL], f32, tag="rb")
            nc.vector.tensor_copy(rb[:], prb[:])

            # scale u by 1/den while copying out of PSUM
            u = smallp.tile([D, BL], f32, tag="u")
            nc.vector.tensor_mul(u[:], pu[:], rb[:])

            # A_lT [l, b] = W_vT.T @ u
            pal = pkp.tile([D, BL], f32, tag="pk")
            nc.tensor.matmul(pal[:], wvT, u[:], start=True, stop=True)
            alT = persist.tile([D, BL], f32)
            nc.vector.tensor_copy(alT[:], pal[:])

            # A_l rows: transpose alT -> [b, l]
            palr = pTp.tile([BL, D], f32, tag="pT")
            nc.tensor.matmul(palr[:], alT[:], if32, is_transpose=True)
            alr = smallp.tile([BL, D], f32, tag="alr")
            nc.vector.tensor_copy(alr[:], palr[:])
            nc.sync.dma_start(al_d[:, :], alr[:])

            # A_lk[b] = transpose(R_T * A_l[b]-col)
            for b in range(BL):
                alkT = outp.tile([D, D], f32, tag="alkT")
                nc.vector.tensor_scalar_mul(alkT[:], rT, alT[:, b : b + 1])
                palk = pkp.tile([D, D], f32, tag="pk")
                nc.tensor.matmul(palk[:], alkT[:], if32, is_transpose=True)
                alk = outp.tile([D, D], f32, tag="alk")
                nc.vector.tensor_copy(alk[:], palk[:])
                nc.gpsimd.dma_start(alk_d.ap()[b], alk[:])

    _split_multi_waits(nc)
    return nc


def _split_multi_waits(nc):
    """This walrus build allows at most ONE sync-wait per instruction
    (setupSyncWait: 'Too many sync wait commands'). Tile attaches one wait
    per dependency semaphore. Split the extras into standalone
    EventSemaphore instructions on the same engine queue, inserted just
    before the instruction — same mechanism Tile's own barriers use."""
    from concourse import mybir

    for fn in nc.m.functions:
        for blk in fn.blocks:
            out = []
            for i in blk.instructions:
                si = getattr(i, "sync_info", None)
                if si is not None and len(si.on_wait) > 1:
                    for j, w in enumerate(si.on_wait[:-1]):
                        out.append(
                            mybir.InstEventSemaphore(
                                name=f"{i.name}_prewait{j}",
                                engine=i.engine,
                                ins=[],
                                outs=[],
                                sync_info=mybir.SyncInfo(on_wait=[w], on_update=[]),
                            )
                        )
                    i.sync_info = mybir.SyncInfo(
                        on_wait=[si.on_wait[-1]], on_update=si.on_update
                    )
                out.append(i)
            try:
                blk.instructions = out
            except Exception:
                blk.instructions.clear()
                blk.instructions.extend(out)


def _get_nc():
    if "nc" not in _CACHE:
        _CACHE["nc"] = _build_nc()
    return _CACHE["nc"]


def kernel(e_i, e_j, imp_j, R_lk, W_q, W_k, W_v, omega):
    from concourse.bass_utils import run_bass_kernel_spmd

    e_i = np.asarray(e_i, np.float32)
    e_j = np.asarray(e_j, np.float32)
    imp_j = np.asarray(imp_j, np.float32)

    wqT = np.asarray(W_q, np.float32).T
    wkT = np.asarray(W_k, np.float32).T.astype(BF16)
    wvT = np.asarray(W_v, np.float32).T
    rT = np.asarray(R_lk, np.float32).T
    om = np.asarray(omega, np.float32).reshape(D, 1).astype(BF16)
    onr = np.ones((1, D), np.float32)
    if32 = np.eye(D, dtype=np.float32)
    cb = np.concatenate([wkT, om], axis=1)

    in_maps = []
    for c in range(NCORES):
        sl = slice(c * BL, (c + 1) * BL)
        impT = imp_j[sl].reshape(BL, 128, T).transpose(1, 2, 0).reshape(128, T * BL)
        cf = np.concatenate(
            [e_i[sl].T, wqT, wvT, rT, if32, np.ones((D, 1), np.float32), impT],
            axis=1,
        ).astype(np.float32)
        in_maps.append(
            {
                "e_j": np.ascontiguousarray(e_j[sl]).astype(BF16),
                "constf": np.ascontiguousarray(cf),
                "constb": np.ascontiguousarray(cb),
                "ones_row": onr,
            }
        )

    nc = _get_nc()
    trace = bool(int(os.environ.get("ATHENA_TRACE", "0")))
    res = run_bass_kernel_spmd(
        nc, in_maps, core_ids=list(range(NCORES)), trace=trace
    )
    _CACHE["last_results"] = res

    A_l = np.concatenate([res.results[c]["A_l"] for c in range(NCORES)], axis=0)
    A_lk = np.concatenate([res.results[c]["A_lk"] for c in range(NCORES)], axis=0)
    return A_l, A_lk, A_l
